# revision 30
# baseline (speedup 1.0000x reference)
"""DecayAttention Trainium2 kernel — 8-core SPMD, bf16 edition.

Problem: B=2, L=2048, D=1024, H=16 heads (Hd=64).
  out = (softmax(Q K^T/sqrt(Hd) - rate_h*log1p(|i-j|) + causal) V) @ Wo.T + bo

Sharding: core c handles batch b = c//4 and heads h in [4*(c%4), 4*(c%4)+4).
Q/K/V projections column-sharded, Wo row-sharded; the 4 cores of each batch
return partial outputs that the host sums (plus Wo@bv + bo, both of which are
q-independent constants because softmax rows sum to 1).

Key layout/perf decisions (v2):
  - All matmul operands in bf16 (PE rate identical to f32r, half the SBUF/DMA
    bytes); PSUM accumulation stays f32.  End-to-end rel-err ~4e-3 (budget 2e-2).
  - x is shipped pre-transposed (xT [D, L]) and column-chunked per q-chunk so
    the first projections unblock after ~2 MB of DMA instead of 11 MB.
  - wq|wk|wv ship as one [D, 768] tensor -> 8 input DMAs instead of 24.
  - scores are computed transposed (S^T[k, q] = K Q^T); V is augmented with a
    ones column so P^T V gives numerator and denominator in one matmul.
  - decay bias + causal mask collapse into one Toeplitz factor
    expA[k, q] = (1+q-k)^(-rate) * [k <= q]; ONE DMA per q-chunk materializes
    all its k-tiles ([128, nkt, 512] windows of a 4095-vector, bf16).
  - exp runs on ACT (PSUM->bf16 SBUF); the expA multiply runs on DVE in the
    4x bf16 mode (all-SBUF, 2-byte, packed); K/Q bias-adds ride the mandatory
    PSUM->SBUF copy on ACT as Identity+bias (same act table set as Exp).
  - softmax normalization: reciprocal of the pair's two denominator rows into
    one [1, 1024] row at partition 64, one broadcast matmul per pair
    (stationary ones at tile_position (64,0)), then per-head muls on DVE.
"""
import math

import numpy as np

import concourse.bass as bass
import concourse.mybir as mybir
import concourse.tile as tile
from concourse import bass_utils

f32 = mybir.dt.float32
f32r = mybir.dt.float32r
bf16 = mybir.dt.bfloat16
Exp = mybir.ActivationFunctionType.Exp
Identity = mybir.ActivationFunctionType.Identity

B, L, D, H = 2, 2048, 1024, 16
Hd = D // H                      # 64
N_CORES = 8
CPB = N_CORES // B               # 4 cores per batch element
HPC = H // CPB                   # 4 heads per core
DHC = HPC * Hd                   # 256 head-dims per core
NQ = L // 512                    # 4 q-chunks of 512
NLT = L // 128                   # 16 l/k tiles of 128
NE = D // 128                    # 8 contraction tiles for projections
GLEN = 2 * L - 1                 # 4095
GOFF = L - 1                     # 2047

# engine-assignment knobs (tuned via cost-model sim)
BIAS_ON_ACT = True       # K/Q bias-add copies: ACT Identity+bias vs DVE
FO_SPLIT = True          # WO output copies: split DVE/ACT vs all-DVE
MUL_POOL_FRAC = 2        # route every Nth expA-mul (h1) to GpSimd; 0 = off
PBC_AT64 = True          # denom broadcast: stationary at PE row 64 vs row 0
EA_BIG_DMA = True        # one ea DMA per q-chunk vs per-kt gpsimd DMAs
H1_DIRECT = True         # odd-head norm-mul writes outT[64:128] directly
MUL_COMBINED = False     # one 0-stride expA mul per pair instead of two
TAIL_SPLIT = False       # last q-chunk: norm muls in column chunks -> WO earlier
START_SPLIT = True       # alternate input DMAs between HWDGE (sync) and SWDGE
P2_BUFS = 5              # p2 staging depth


def _split_multi_waits(nc):
    """This container's walrus accepts at most one sync-wait per engine
    instruction; hoist extras onto single-wait NOPs placed just before."""
    for fn in nc.m.functions:
        for blk in fn.blocks:
            out, changed = [], False
            for inst in blk.instructions:
                si = inst.sync_info
                waits = list(si.on_wait) if si is not None and si.on_wait else []
                if len(waits) > 1:
                    changed = True
                    for w in waits[:-1]:
                        nop = mybir.InstNoOp(
                            name=nc.get_next_instruction_name(), ins=[], outs=[])
                        nop.engine = inst.engine
                        nop.sync_info = mybir.SyncInfo(on_wait=[w], on_update=[])
                        out.append(nop)
                    inst.sync_info = mybir.SyncInfo(
                        on_wait=[waits[-1]], on_update=list(si.on_update or []))
                out.append(inst)
            if changed:
                blk.instructions = out


def build_nc(n_g: int, phases=("A", "B", "WO"), repeat=1, internal_io=False):
    """Build the per-core Bass program. n_g = 1 (all heads share one decay
    rate, the setup_inputs case) or HPC (per-head expA vectors)."""
    nc = bass.Bass("TRN2", target_bir_lowering=False, debug=False)

    big = "Internal" if internal_io else "ExternalInput"
    xT = nc.dram_tensor("xT", [D, L], bf16, kind=big).ap()
    wqkv = nc.dram_tensor("wqkv", [D, 3 * DHC], bf16, kind=big).ap()
    woT = nc.dram_tensor("woT", [DHC, D], bf16, kind=big).ap()
    bqk = nc.dram_tensor("bqk", [DHC, 2], f32, kind="ExternalInput").ap()
    g = nc.dram_tensor("g", [n_g, GLEN], bf16, kind="ExternalInput")
    pmask = nc.dram_tensor("pmask", [128, NLT], f32, kind="ExternalInput").ap()
    out = nc.dram_tensor(
        "out", [L, D], bf16,
        kind="Internal" if internal_io else "ExternalOutput").ap()
    tok = (nc.dram_tensor("tok", [128, 2], bf16, kind="ExternalOutput").ap()
           if internal_io else None)

    NP = HPC // 2                # head pairs per core

    with tile.TileContext(nc) as tc:
      for _rep in range(repeat):
        with tc.tile_pool(name="cons", bufs=1) as cons:
            # persistent SBUF residents (pair layout: pair p = heads 2p, 2p+1)
            qt_p = [cons.tile([128, L], bf16, name=f"qt{p}") for p in range(NP)]
            kt_p = [cons.tile([128, L], bf16, name=f"kt{p}") for p in range(NP)]
            vaug = [cons.tile([128, 65 * HPC], bf16, name=f"vaug{t}")
                    for t in range(NLT)]
            wo_p = [cons.tile([128, D], bf16, name=f"wo{p}") for p in range(NP)]
            bq_p = [cons.tile([128, 1], f32, name=f"bq{p}") for p in range(NP)]
            bk_p = [cons.tile([128, 1], f32, name=f"bk{p}") for p in range(NP)]
            pm2 = cons.tile([128, NLT], f32)
            ones64 = cons.tile([128, Hd], f32r)   # only row 64 used
            ones_st = cons.tile([128, HPC], f32)

            ones_sf = cons.tile([128, Hd], f32)
            nc.vector.memset(ones_st[:, :], 1.0)
            nc.vector.memset(ones_sf[:, :], 1.0)
            nc.vector.tensor_copy(ones64[:, :], ones_sf[:, :])
            # early small inputs on the scalar queue (ACT idle during load)
            nc.scalar.dma_start(pm2[:, :], pmask)
            for p in range(NP):
                nc.scalar.dma_start(bq_p[p][:, :],
                                    bqk[p * 128:(p + 1) * 128, 0:1])
                nc.scalar.dma_start(bk_p[p][:, :],
                                    bqk[p * 128:(p + 1) * 128, 1:2])
            for p in range(NP):
                nc.scalar.dma_start(wo_p[p][:, :], woT[p * 128:(p + 1) * 128, :])

            with tc.tile_pool(name="eap", bufs=2) as eap, \
                 tc.tile_pool(name="wrk", bufs=3) as wrk, \
                 tc.tile_pool(name="otp", bufs=2) as otp, \
                 tc.tile_pool(name="psS", bufs=2, space="PSUM") as psS, \
                 tc.tile_pool(name="psV", bufs=HPC, space="PSUM") as psV, \
                 tc.tile_pool(name="xw", bufs=1) as xw:
                psA = psS
                # x in per-qc column chunks so qc=0 compute starts early
                xt_t = [[xw.tile([128, 512], bf16, name=f"x{e}_{c}")
                         for c in range(NQ)] for e in range(NE)]
                wq_t = [None] * NE
                wk_t = [None] * NE
                wv_t = [None] * NE
                wqkv_t = []
                for e in range(NE):
                    t3 = xw.tile([128, 3 * DHC], bf16, name=f"w3{e}")
                    wqkv_t.append(t3)
                    wq_t[e] = t3[:, 0:DHC]
                    wk_t[e] = t3[:, DHC:2 * DHC]
                    wv_t[e] = t3[:, 2 * DHC:3 * DHC]
                def in_eng(i):
                    return (nc.gpsimd if (START_SPLIT and i % 2)
                            else nc.sync)
                for e in range(NE):
                    in_eng(e).dma_start(wqkv_t[e][:, :],
                                        wqkv[e * 128:(e + 1) * 128, :])
                for e in range(NE):
                    in_eng(e + 1).dma_start(xt_t[e][0][:, :],
                                            xT[e * 128:(e + 1) * 128, 0:512])

                # ea windows: one DMA per q-chunk brings every k-tile's
                # [128, 512] reversed expA slab: ea_all[qc][k, kt, j] =
                # g[GOFF + kt*128 - q0 - 511 + k + j]  (j = 511 - (q - q0))
                ea_all = [None] * NQ

                def ea_dma(qc):
                    nkt = 4 * (qc + 1)
                    t = eap.tile([128, n_g * nkt * 512], bf16, name="ea",
                                 tag="ea")
                    for r in range(n_g):
                        nc.gpsimd.dma_start(
                            bass.AP(t.tensor,
                                    t[:, :].offset + r * nkt * 512,
                                    [[t[:, :].ap[0][0], 128],
                                     [512, nkt], [1, 512]]),
                            bass.AP(g, r * GLEN + GOFF - qc * 512 - 511,
                                    [[1, 128], [128, nkt], [1, 512]]))
                    return t

                def v_tile(t):
                    pv = psA.tile([128, DHC], f32, name="pv", tag="s")
                    for e in range(NE):
                        nc.tensor.matmul(
                            pv[:, :], xt_t[e][t // 4][:, (t % 4) * 128:
                                                      (t % 4 + 1) * 128],
                            wv_t[e][:, :],
                            start=(e == 0), stop=(e == NE - 1))
                    dst = bass.AP(vaug[t].tensor, vaug[t][:, :].offset,
                                  [[vaug[t][:, :].ap[0][0], 128],
                                   [65, HPC], [1, Hd]])
                    src_ = bass.AP(pv.tensor, pv[:, :].offset,
                                   [[pv[:, :].ap[0][0], 128], [Hd, HPC],
                                    [1, Hd]])
                    with nc.allow_low_precision(reason="bf16 V for PE"):
                        nc.vector.tensor_scalar_mul(dst, src_, pm2[:, t:t + 1])
                        ones_dst = bass.AP(vaug[t].tensor,
                                           vaug[t][:, :].offset + Hd,
                                           [[vaug[t][:, :].ap[0][0], 128],
                                            [65, HPC]])
                        nc.vector.tensor_scalar_mul(ones_dst, ones_st[:, :],
                                                    pm2[:, t:t + 1])

                def kq_bias(dst, src, bias):
                    with nc.allow_low_precision(reason="bf16 K/Q for PE"):
                        if BIAS_ON_ACT:
                            nc.scalar.activation(dst, src, Identity,
                                                 bias=bias)
                        else:
                            nc.vector.tensor_scalar_add(dst, src, bias)

                def proj(qc):
                    """V tiles + K/Q projections for chunk qc."""
                    for t in range(4 * qc, 4 * qc + 4):
                        v_tile(t)
                    for p in range(NP):
                        ps_ = p * 128
                        pk = psA.tile([128, 512], f32, name="pk", tag="s")
                        for e in range(NE):
                            nc.tensor.matmul(
                                pk[:, :], wk_t[e][:, ps_:ps_ + 128],
                                xt_t[e][qc][:, :],
                                start=(e == 0), stop=(e == NE - 1))
                        kq_bias(kt_p[p][:, qc * 512:(qc + 1) * 512], pk[:, :],
                                bk_p[p][:, 0:1])
                        pq = psA.tile([128, 512], f32, name="pq", tag="s")
                        for e in range(NE):
                            nc.tensor.matmul(
                                pq[:, :], wq_t[e][:, ps_:ps_ + 128],
                                xt_t[e][qc][:, :],
                                start=(e == 0), stop=(e == NE - 1))
                        kq_bias(qt_p[p][:, qc * 512:(qc + 1) * 512], pq[:, :],
                                bq_p[p][:, 0:1])

            # ---- per q-chunk: projections + attention + output ----
                if "B" in phases or "P" in phases:
                    ea_all[0] = ea_dma(0)
                    proj(0)
                for qc in range(NQ if "B" in phases else 0):
                    # prefetch next chunk's x columns and ea slabs
                    if qc + 1 < NQ:
                        for e in range(NE):
                            in_eng(e + qc).dma_start(
                                xt_t[e][qc + 1][:, :],
                                xT[e * 128:(e + 1) * 128,
                                   (qc + 1) * 512:(qc + 2) * 512])
                        ea_all[qc + 1] = ea_dma(qc + 1)

                    q0 = qc * 512
                    nkt = (qc + 1) * (NLT // NQ)
                    pvh = [psV.tile([65, 512], f32, name="pvh", tag="pvh")
                           for _ in range(HPC)]
                    outT_p = [otp.tile([128, 512], bf16, name="otp",
                                       tag=f"otp{p}") for p in range(NP)]
                    ea_t = ea_all[qc]
                    ea_pitch = ea_t[:, :].ap[0][0]
                    ea_base = ea_t[:, :].offset
                    for kt in range(nkt):
                        # q-columns below this k-tile's diagonal are exact
                        # zeros in expA — trim S/exp/mul/PV to the live range.
                        qlo = max(0, kt * 128 - q0)
                        nn_ = 512 - qlo
                        for pr in range(NP):
                            h0, h1 = 2 * pr, 2 * pr + 1
                            ps2 = psS.tile([128, 1024], f32, name="ps2",
                                           tag="s")
                            nc.tensor.matmul(
                                ps2[:, qlo:512],
                                kt_p[pr][0:64, kt * 128:(kt + 1) * 128],
                                qt_p[pr][0:64, q0 + qlo:q0 + 512],
                                start=True, stop=True, tile_position=(0, 0))
                            nc.tensor.matmul(
                                ps2[:, 512 + qlo:1024],
                                kt_p[pr][64:128, kt * 128:(kt + 1) * 128],
                                qt_p[pr][64:128, q0 + qlo:q0 + 512],
                                start=True, stop=True, tile_position=(64, 0))
                            p2 = wrk.tile([128, 1024], bf16, name="p2",
                                          bufs=P2_BUFS)
                            exp_in = bass.AP(ps2.tensor, ps2[:, :].offset + qlo,
                                             [[1024, 128], [512, 2], [1, nn_]])
                            exp_out = bass.AP(p2.tensor, p2[:, :].offset + qlo,
                                              [[1024, 128], [512, 2], [1, nn_]])
                            with nc.allow_low_precision(
                                    reason="exp output feeds bf16 PV matmul"):
                                nc.scalar.activation(
                                    exp_out, exp_in, Exp,
                                    scale=(0.0 if internal_io else 1.0))
                            if MUL_COMBINED and n_g == 1:
                                # one mul covers both heads: ea repeated via
                                # 0-stride dim, reversed along q
                                rev2 = bass.AP(
                                    ea_t.tensor,
                                    ea_base + kt * 512 + 511 - qlo,
                                    [[ea_pitch, 128], [0, 2], [-1, nn_]])
                                both = bass.AP(
                                    p2.tensor, p2[:, :].offset + qlo,
                                    [[1024, 128], [512, 2], [1, nn_]])
                                mul_eng = (nc.gpsimd
                                           if (MUL_POOL_FRAC and
                                               (kt * NP + pr) % MUL_POOL_FRAC
                                               == 0)
                                           else nc.vector)
                                with nc.allow_low_precision(
                                        reason="bf16 attention weights"):
                                    mul_eng.tensor_mul(both, both, rev2)
                            for hh, hoff in ((h0, 0), (h1, 512)):
                                if not (MUL_COMBINED and n_g == 1):
                                    # reversed-q read of this kt's expA slab
                                    rev = bass.AP(
                                        ea_t.tensor,
                                        ea_base + ((hh % n_g) * nkt + kt) * 512
                                        + 511 - qlo,
                                        [[ea_pitch, 128], [-1, nn_]])
                                    mul_eng = (nc.gpsimd
                                               if (MUL_POOL_FRAC and
                                                   (kt * NP + pr)
                                                   % MUL_POOL_FRAC
                                                   == 0 and hh == h1)
                                               else nc.vector)
                                    with nc.allow_low_precision(
                                            reason="bf16 attention weights"):
                                        mul_eng.tensor_mul(
                                            p2[:, hoff + qlo:hoff + 512],
                                            p2[:, hoff + qlo:hoff + 512], rev)
                                nc.tensor.matmul(
                                    pvh[hh][:, qlo:512],
                                    vaug[kt][:, 65 * hh:65 * hh + 65],
                                    p2[:, hoff + qlo:hoff + 512],
                                    start=(kt == 0), stop=(kt == nkt - 1))

                    if qc + 1 < NQ:
                        proj(qc + 1)

                    # normalization: reciprocal of each pair's two denominator
                    # rows into one [1, 1024] row at partition 64, then one
                    # broadcast matmul per pair, then per-head muls.
                    for pr in range(NP):
                        h0, h1 = 2 * pr, 2 * pr + 1
                        rp = 64 if PBC_AT64 else 0
                        tp = {"tile_position": (64, 0)} if PBC_AT64 else {}
                        rec = wrk.tile([128, 1024], f32r, name="rec", tag="rec")
                        with nc.allow_low_precision(
                                reason="softmax denom reciprocal"):
                            nc.vector.reciprocal(rec[rp:rp + 1, 0:512],
                                                 pvh[h0][64:65, :])
                            nc.vector.reciprocal(rec[rp:rp + 1, 512:1024],
                                                 pvh[h1][64:65, :])
                        pbc = psS.tile([64, 1024], f32, name="pbc", tag="s")
                        nc.tensor.matmul(pbc[:, 0:512],
                                         ones64[rp:rp + 1, :],
                                         rec[rp:rp + 1, 0:512], start=True,
                                         stop=True, **tp)
                        nc.tensor.matmul(pbc[:, 512:1024],
                                         ones64[rp:rp + 1, :],
                                         rec[rp:rp + 1, 512:1024], start=True,
                                         stop=True, **tp)
                        # DVE reads at most one PSUM operand: stage the
                        # broadcast to SBUF (on ACT — idle at chunk ends)
                        bc = wrk.tile([64, 1024], f32r, name="bc", tag="bc")
                        ncs = 4 if (TAIL_SPLIT and qc == NQ - 1) else 1
                        cw = 512 // ncs
                        with nc.allow_low_precision(reason="bf16 attn out"):
                            nc.scalar.activation(bc[:, :], pbc[:, :], Identity)
                            for ci in range(ncs):
                                c0, c1 = ci * cw, (ci + 1) * cw
                                nc.vector.tensor_mul(
                                    outT_p[pr][0:64, c0:c1],
                                    pvh[h0][0:64, c0:c1], bc[:, c0:c1])
                                if H1_DIRECT:
                                    nc.vector.tensor_mul(
                                        outT_p[pr][64:128, c0:c1],
                                        pvh[h1][0:64, c0:c1],
                                        bc[:, 512 + c0:512 + c1])
                            if not H1_DIRECT:
                                ostg = wrk.tile([64, 512], bf16, name="ostg",
                                                tag="fo")
                                nc.vector.tensor_mul(
                                    ostg[:, :], pvh[h1][0:64, :],
                                    bc[:, 512:1024])
                                nc.sync.dma_start(outT_p[pr][64:128, :],
                                                  ostg[:, :])

                    for m in range(4 if "WO" in phases else 0):
                        fo = wrk.tile([128, 1024], bf16, name="fo2", tag="fo2")
                        for n in range(2):
                            pf = psV.tile([128, 512], f32, name="pf",
                                          tag="pvh")
                            for p in range(NP):
                                nc.tensor.matmul(
                                    pf[:, :],
                                    outT_p[p][:, m * 128:(m + 1) * 128],
                                    wo_p[p][:, n * 512:(n + 1) * 512],
                                    start=(p == 0), stop=(p == NP - 1))
                            with nc.allow_low_precision(reason="bf16 out"):
                                if FO_SPLIT and n == 1:
                                    nc.scalar.activation(
                                        fo[:, 512:1024], pf[:, :], Identity)
                                else:
                                    nc.vector.tensor_copy(
                                        fo[:, n * 512:(n + 1) * 512],
                                        pf[:, :])
                        nc.sync.dma_start(
                            out[q0 + m * 128:q0 + (m + 1) * 128, :],
                            fo[:, :])
                        if internal_io and qc == NQ - 1 and m == 3:
                            nc.sync.dma_start(tok, fo[:, 0:2])

    _split_multi_waits(nc)
    return nc


_NC_CACHE = {}
_last_in_maps = None
_last_n_g = 1


def _get_nc(n_g):
    if n_g not in _NC_CACHE:
        _NC_CACHE[n_g] = build_nc(n_g)
    return _NC_CACHE[n_g]


def _np_bf16():
    import ml_dtypes
    return ml_dtypes.bfloat16


def kernel(x, causal_mask, key_padding_mask, Wq, bq, Wk, bk, Wv, bv, Wo, bo,
           decay_logit):
    nbf = _np_bf16()
    x = np.asarray(x, dtype=np.float32)
    Wq = np.asarray(Wq, dtype=np.float32)
    Wk = np.asarray(Wk, dtype=np.float32)
    Wv = np.asarray(Wv, dtype=np.float32)
    Wo = np.asarray(Wo, dtype=np.float32)
    bq = np.asarray(bq, dtype=np.float32)
    bk = np.asarray(bk, dtype=np.float32)
    bv = np.asarray(bv, dtype=np.float32)
    bo = np.asarray(bo, dtype=np.float32)
    decay_logit = np.asarray(decay_logit, dtype=np.float32)
    key_padding_mask = np.asarray(key_padding_mask)

    scale = 1.0 / math.sqrt(Hd)
    rates = np.log1p(np.exp(decay_logit.astype(np.float64)))  # softplus [H]

    def g_vec(rate):
        d = np.arange(GLEN) - GOFF           # d = q - k in [-2047, 2047]
        vals = np.where(d >= 0, (1.0 + np.abs(d)) ** (-rate), 0.0)
        # device AP reads g[GOFF + k - q] => store reversed
        return vals[::-1].astype(nbf)

    in_maps = []
    n_g_needed = 1
    for c in range(N_CORES):
        b = c // CPB
        hs = (c % CPB) * HPC                 # first head of this core
        sl = slice(hs * Hd, (hs + HPC) * Hd)
        core_rates = rates[hs:hs + HPC]
        if not np.allclose(core_rates, core_rates[0], rtol=1e-6, atol=1e-9):
            n_g_needed = HPC
        gmat = (np.stack([g_vec(core_rates[0])])
                if n_g_needed == 1
                else np.stack([g_vec(r) for r in core_rates]))
        wq_s = (Wq[sl] * scale).T            # [D, DHC]
        wk_s = Wk[sl].T
        wv_s = Wv[sl].T
        in_maps.append({
            "xT": np.ascontiguousarray(x[b].T.astype(nbf)),
            "wqkv": np.ascontiguousarray(
                np.concatenate([wq_s, wk_s, wv_s], axis=1).astype(nbf)),
            "woT": np.ascontiguousarray(Wo[:, sl].T.astype(nbf)),
            "bqk": np.ascontiguousarray(
                np.stack([bq[sl] * scale, bk[sl]], axis=1)),
            "g": gmat,
            "pmask": np.ascontiguousarray(
                (~key_padding_mask[b]).astype(np.float32)
                .reshape(NLT, 128).T),
        })

    global _last_in_maps, _last_n_g
    _last_in_maps, _last_n_g = in_maps, n_g_needed
    nc = _get_nc(n_g_needed)
    res = bass_utils.run_bass_kernel_spmd(
        nc, in_maps, core_ids=list(range(N_CORES)))

    # q-independent constant: Wo @ bv + bo (softmax rows sum to 1)
    const = Wo.astype(np.float64) @ bv.astype(np.float64) + bo
    out = np.zeros((B, L, D), dtype=np.float64)
    for c in range(N_CORES):
        out[c // CPB] += np.asarray(res.results[c]["out"], dtype=np.float64)
    out += const[None, None, :]
    return out.astype(np.float32)
